# revision 2
# baseline (speedup 1.0000x reference)
"""Trainium2 Bass kernel: grouped full attention with dynamic relative
position bias (8 heads, 400 tokens/group, dim 256, batch 128).

Data parallel over the 128 (batch*group) rows - 16 per core.  The tiny
position-bias MLP runs on host; its [400,400] per-head bias table is
factored as rank-32 F G^T and folded INTO the score matmul as a second
accumulation pass with constant fp/bf16 factors.  This removes the
1.28M-element/group rb-multiply entirely: ACT exps straight out of PSUM.

Per group g (m = key token, n = query token):
  phase A: qkT = Wqk^T x^T   (PSUM, 4 banks) -> kq sbuf bf16
  phase B: v   = x Wv        (PSUM)          -> v sbuf bf16 (+tail gather v3)
  phase C: AV(g-1): U^T = V^T E, sums = 1^T E (PSUM, accumulate over m)
           recS = 1/sums, unT = U^T * recS
  phase D: scores S^T[m,n] = K Q^T + F G^T (2-pass accumulate, 4-head
           row packing), exp on ACT (FD=1600 strided over 4 banks) -> E bf16
  phase E: proj(g-1): out = unT^T Wp -> DMA
All phases rotate through one 2-buffer [128,4,512] PSUM pool (8 banks).
"""

import numpy as np
import ml_dtypes

import concourse.bass as bass
import concourse.mybir as mybir
import concourse.tile as tile
from concourse import bacc
from concourse.bass import ts
from concourse.bass_utils import run_bass_kernel_spmd

T, V = 16, 25
N = T * V              # 400
DIM = 256
HEADS = 8
HEAD_DIM = 32
SCALE = HEAD_DIM ** -0.5
LN_EPS = 1e-5
B_ = 128
NCORES = 8
BPC = B_ // NCORES
RANK = 32              # rpb low-rank fold

F32 = mybir.dt.float32
BF16 = mybir.dt.bfloat16

# m-chunks: three 128/128/112 plus a 32-row diagonal tail (368:400)
MC = [(0, 128), (128, 128), (256, 112)]
TAIL_OFF = 368

_CACHE = {}


def _pos_mlp_host(posproj_w, posproj_b, ln1_g, ln1_b, p1_w, p1_b,
                  ln2_g, ln2_b, p2_w, p2_b, ln3_g, ln3_b, p3_w, p3_b):
    bh = np.arange(1 - T, T, dtype=np.float32)
    bw = np.arange(1 - V, V, dtype=np.float32)
    grid = np.stack(np.meshgrid(bh, bw, indexing="ij"))
    biases = grid.reshape(2, -1).T.astype(np.float32)

    def layernorm(x, g, b):
        mu = x.mean(axis=-1, keepdims=True)
        var = x.var(axis=-1, keepdims=True)
        return (x - mu) / np.sqrt(var + LN_EPS) * g + b

    pos = biases @ posproj_w + posproj_b
    pos = np.maximum(layernorm(pos, ln1_g, ln1_b), 0.0) @ p1_w + p1_b
    pos = np.maximum(layernorm(pos, ln2_g, ln2_b), 0.0) @ p2_w + p2_b
    pos = np.maximum(layernorm(pos, ln3_g, ln3_b), 0.0) @ p3_w + p3_b
    return pos.astype(np.float32)


def _rel_idx_host():
    coords = np.stack(np.meshgrid(np.arange(T), np.arange(V), indexing="ij"))
    cf = coords.reshape(2, -1)
    rel = (cf[:, :, None] - cf[:, None, :]).transpose(1, 2, 0)
    rel[:, :, 0] += T - 1
    rel[:, :, 1] += V - 1
    rel[:, :, 0] *= 2 * V - 1
    return rel.sum(-1).astype(np.int32)


def _emit(ctx, tc, d, flags, bpc):
    nc = tc.nc
    bqk_nonzero, vb_nonzero, pb_nonzero = flags

    const = ctx.enter_context(tc.tile_pool(name="const", bufs=1))
    xt_pool = ctx.enter_context(tc.tile_pool(name="xt", bufs=3))
    kq_pool = ctx.enter_context(tc.tile_pool(name="kq", bufs=2))
    v_pool = ctx.enter_context(tc.tile_pool(name="v", bufs=2))
    e_pool = ctx.enter_context(tc.tile_pool(name="e", bufs=8))
    et_pool = ctx.enter_context(tc.tile_pool(name="et", bufs=2))
    rs_pool = ctx.enter_context(tc.tile_pool(name="rs", bufs=3))
    un_pool = ctx.enter_context(tc.tile_pool(name="un", bufs=3))
    o_pool = ctx.enter_context(tc.tile_pool(name="o", bufs=2))
    psA = ctx.enter_context(tc.tile_pool(name="psA", bufs=2, space="PSUM"))

    w_qk = const.tile([128, 2, 512], BF16)
    nc.sync.dma_start(w_qk[:], d["w_qk"][:])
    w_v = const.tile([128, 2, 256], BF16)
    nc.sync.dma_start(w_v[:], d["w_v"][:])
    w_p = const.tile([128, 2, 256], BF16)
    nc.sync.dma_start(w_p[:], d["w_p"][:])
    ft = const.tile([128, 2, N], BF16)
    nc.sync.dma_start(ft[:], d["ft"][:])
    gt = const.tile([128, 2, N], BF16)
    nc.sync.dma_start(gt[:], d["gt"][:])
    ones = const.tile([128, 32], BF16)
    nc.vector.memset(ones[:], 1.0)
    if bqk_nonzero:
        bqk = const.tile([128, 4], F32)
        nc.sync.dma_start(bqk[:], d["bqk"][:])
    if vb_nonzero:
        vb = const.tile([128, 256], F32)
        nc.sync.dma_start(
            vb[:],
            bass.AP(tensor=d["bv"].tensor, offset=d["bv"].offset,
                    ap=[[0, 128]] + d["bv"].ap),
        )
    if pb_nonzero:
        pb = const.tile([128, 256], F32)
        nc.sync.dma_start(
            pb[:],
            bass.AP(tensor=d["bp"].tensor, offset=d["bp"].offset,
                    ap=[[0, 128]] + d["bp"].ap),
        )

    def emit_qkv(b):
        xt = xt_pool.tile([128, 2, N], BF16)
        nc.sync.dma_start(xt[:], d["xt"][b])
        ps = psA.tile([128, 4, 512], F32, tag="ps")
        for t in range(4):
            for cc in range(2):
                nc.tensor.matmul(
                    ps[:, t, 0:N], w_qk[:, cc, ts(t, 128)], xt[:, cc, :],
                    start=(cc == 0), stop=(cc == 1),
                )
        kq = kq_pool.tile([128, 4, N], BF16)
        if bqk_nonzero:
            for t in range(4):
                nc.vector.tensor_scalar(
                    out=kq[:, t, :], in0=ps[:, t, 0:N],
                    scalar1=bqk[:, t:t + 1], scalar2=None,
                    op0=mybir.AluOpType.add)
        else:
            nc.vector.tensor_copy(out=kq[:, :, :], in_=ps[:, :, 0:N])

        ps2 = psA.tile([128, 4, 512], F32, tag="ps")
        for nt in range(4):
            off, m = nt * 128, (128 if nt < 3 else N - 384)
            for cc in range(2):
                nc.tensor.matmul(
                    ps2[0:m, nt, 0:256], xt[:, cc, off:off + m], w_v[:, cc, :],
                    start=(cc == 0), stop=(cc == 1),
                )
        v = v_pool.tile([128, 4, 256], BF16)
        if vb_nonzero:
            for nt in range(4):
                m = 128 if nt < 3 else N - 384
                nc.vector.tensor_tensor(
                    out=v[0:m, nt, :], in0=ps2[0:m, nt, 0:256],
                    in1=vb[0:m, :], op=mybir.AluOpType.add)
        else:
            nc.vector.tensor_copy(out=v[:, :, :], in_=ps2[:, :, 0:256])
        # tail rows 368:400 gathered to diagonal strips: v3[32j:32j+16] =
        # v rows 368:384 (= v[112:128, 2]), v3[32j+16:32j+32] = 384:400
        v3 = v_pool.tile([128, 256], BF16, tag="v3")
        for j in range(4):
            nc.sync.dma_start(v3[32 * j:32 * j + 16, :], v[112:128, 2, :])
            nc.sync.dma_start(v3[32 * j + 16:32 * j + 32, :], v[0:16, 3, :])
        return kq, v, v3

    def emit_scores(b, kq):
        """Scores+exp for group b: 6 (q,mc) slots + merged diagonal tail."""
        E = {}
        for q in range(2):
            for mc in range(3):
                off, m = MC[mc]
                ps = psA.tile([128, 4, 512], F32, tag="ps")
                for a in range(4):
                    nc.tensor.matmul(
                        ps[0:m, a, 0:N],
                        kq[ts(a, 32), 2 + q, off:off + m],
                        kq[ts(a, 32), q, :],
                        start=True, stop=False,
                        tile_position=(32 * a, 0),
                    )
                for a in range(4):
                    nc.tensor.matmul(
                        ps[0:m, a, 0:N],
                        ft[ts(a, 32), q, off:off + m],
                        gt[ts(a, 32), q, :],
                        start=False, stop=True,
                        tile_position=(32 * a, 0),
                        skip_group_check=True,
                    )
                e = e_pool.tile([128, 4, N], BF16, tag="e")
                nc.scalar.activation(
                    out=e[0:m, :, :], in_=ps[0:m, :, 0:N],
                    func=mybir.ActivationFunctionType.Exp,
                )
                E[(q, mc)] = e
        # tail m=368:400, head 4q+j at (row 32j, col 32j), banks q
        ps = psA.tile([128, 4, 512], F32, tag="ps")
        for q in range(2):
            for j in range(4):
                nc.tensor.matmul(
                    ps[ts(j, 32), q, 0:N],
                    kq[ts(j, 32), 2 + q, TAIL_OFF:N],
                    kq[ts(j, 32), q, :],
                    start=True, stop=False,
                    tile_position=(32 * j, 32 * j),
                    skip_group_check=True,
                )
                nc.tensor.matmul(
                    ps[ts(j, 32), q, 0:N],
                    ft[ts(j, 32), q, TAIL_OFF:N],
                    gt[ts(j, 32), q, :],
                    start=False, stop=True,
                    tile_position=(32 * j, 32 * j),
                    skip_group_check=True,
                )
        et = et_pool.tile([128, 2, N], BF16, tag="et")
        nc.scalar.activation(
            out=et[:, :, :], in_=ps[:, 0:2, 0:N],
            func=mybir.ActivationFunctionType.Exp,
        )
        E["tail"] = et
        return E

    def emit_av(q, Eprev, vprev, v3prev):
        """U^T and sums for quad q of the previous group; returns unT[128,N]."""
        ps = psA.tile([128, 4, 512], F32, tag="ps")
        for mc in range(3):
            off, k = MC[mc]
            for bank in range(2):
                for a in range(4):
                    h = 4 * q + a
                    lhs = (vprev[0:k, mc, ts(h, 32)] if bank == 0
                           else ones[0:k, :])
                    nc.tensor.matmul(
                        ps[ts(a, 32), bank, 0:N],
                        lhs,
                        Eprev[(q, mc)][0:k, a, :],
                        start=(mc == 0), stop=False,
                        tile_position=(0, 32 * a),
                        skip_group_check=True,
                    )
        etp = Eprev["tail"]
        for bank in range(2):
            for j in range(4):
                h = 4 * q + j
                lhs = (v3prev[ts(j, 32), ts(h, 32)] if bank == 0
                       else ones[ts(j, 32), :])
                nc.tensor.matmul(
                    ps[ts(j, 32), bank, 0:N],
                    lhs,
                    etp[ts(j, 32), q, :],
                    start=False, stop=True,
                    tile_position=(32 * j, 32 * j),
                    skip_group_check=True,
                )
        recS = rs_pool.tile([128, N], F32, tag="rs")
        nc.vector.reciprocal_approx_fast(out=recS[:], in_=ps[:, 1, 0:N])
        un = un_pool.tile([128, N], BF16, tag="un")
        nc.vector.tensor_tensor(
            out=un[:], in0=ps[:, 0, 0:N], in1=recS[:],
            op=mybir.AluOpType.mult,
        )
        return un

    def emit_proj(b, un0, un1):
        ps = psA.tile([128, 4, 512], F32, tag="ps")
        for nt in range(4):
            off, m = nt * 128, (128 if nt < 3 else N - 384)
            for cc, un in ((0, un0), (1, un1)):
                nc.tensor.matmul(
                    ps[0:m, nt, 0:256], un[:, off:off + m], w_p[:, cc, :],
                    start=(cc == 0), stop=(cc == 1),
                )
        o = o_pool.tile([128, 4, 256], F32)
        if pb_nonzero:
            for nt in range(4):
                m = 128 if nt < 3 else N - 384
                nc.vector.tensor_tensor(
                    out=o[0:m, nt, :], in0=ps[0:m, nt, 0:256], in1=pb[0:m, :],
                    op=mybir.AluOpType.add)
        else:
            nc.vector.tensor_copy(out=o[:, :, :], in_=ps[:, :, 0:256])
        for nt in range(4):
            m = 128 if nt < 3 else N - 384
            nc.sync.dma_start(d["out"][b, nt * 128:nt * 128 + m], o[0:m, nt, :])

    prev = None  # (E, v, v3) of group b-1
    un_hold = None
    for b in range(bpc):
        kq, v, v3 = emit_qkv(b)
        if prev is not None:
            un0 = emit_av(0, *prev)
            un1 = emit_av(1, *prev)
            un_hold = (b - 1, un0, un1)
        E = emit_scores(b, kq)
        if un_hold is not None:
            emit_proj(un_hold[0], un_hold[1], un_hold[2])
            un_hold = None
        prev = (E, v, v3)
    un0 = emit_av(0, *prev)
    un1 = emit_av(1, *prev)
    emit_proj(bpc - 1, un0, un1)


def _build(flags, bpc=BPC):
    nc = bacc.Bacc("TRN2", target_bir_lowering=False, debug=False,
                   num_devices=NCORES)
    bqk_nonzero, vb_nonzero, pb_nonzero = flags
    d = {}
    d["xt"] = nc.dram_tensor("xt", [bpc, 128, 2, N], BF16,
                             kind="ExternalInput").ap()
    d["w_qk"] = nc.dram_tensor("w_qk", [128, 2, 512], BF16,
                               kind="ExternalInput").ap()
    d["w_v"] = nc.dram_tensor("w_v", [128, 2, 256], BF16,
                              kind="ExternalInput").ap()
    d["w_p"] = nc.dram_tensor("w_p", [128, 2, 256], BF16,
                              kind="ExternalInput").ap()
    d["ft"] = nc.dram_tensor("ft", [128, 2, N], BF16,
                             kind="ExternalInput").ap()
    d["gt"] = nc.dram_tensor("gt", [128, 2, N], BF16,
                             kind="ExternalInput").ap()
    if bqk_nonzero:
        d["bqk"] = nc.dram_tensor("bqk", [128, 4], F32,
                                  kind="ExternalInput").ap()
    if vb_nonzero:
        d["bv"] = nc.dram_tensor("bv", [256], F32, kind="ExternalInput").ap()
    if pb_nonzero:
        d["bp"] = nc.dram_tensor("bp", [256], F32, kind="ExternalInput").ap()
    d["out"] = nc.dram_tensor("out", [bpc, N, DIM], F32,
                              kind="ExternalOutput").ap()

    from contextlib import ExitStack

    with tile.TileContext(nc) as tc:
        with ExitStack() as ctx:
            _emit(ctx, tc, d, flags, bpc)
    nc.compile()
    return nc, d


def _prep_host(inputs):
    x = np.ascontiguousarray(np.asarray(inputs["x"], dtype=np.float32))
    qkv_w = np.asarray(inputs["qkv_w"], dtype=np.float32)
    qkv_b = np.asarray(inputs["qkv_b"], dtype=np.float32)
    proj_w = np.asarray(inputs["proj_w"], dtype=np.float32)
    proj_b = np.asarray(inputs["proj_b"], dtype=np.float32)

    pos = _pos_mlp_host(
        *[np.asarray(inputs[k], dtype=np.float32) for k in (
            "posproj_w", "posproj_b", "ln1_g", "ln1_b", "p1_w", "p1_b",
            "ln2_g", "ln2_b", "p2_w", "p2_b", "ln3_g", "ln3_b",
            "p3_w", "p3_b")])
    rel = _rel_idx_host()
    rpb_nmh = pos[rel.reshape(-1)].reshape(N, N, HEADS)   # [n, m, h]

    # rank-RANK factorization of each head's [m, n] bias: rpbT ~= F @ G^T
    ft_dev = np.zeros((128, 2, N), np.float32)   # [32a+r, q, m] = F_{4q+a}[m,r]
    gt_dev = np.zeros((128, 2, N), np.float32)
    for h in range(HEADS):
        Mh = rpb_nmh[:, :, h].T                  # [m, n]
        U, s, Vt = np.linalg.svd(Mh, full_matrices=False)
        r = RANK
        F = U[:, :r] * np.sqrt(s[:r])            # [m, r]
        G = (Vt[:r].T * np.sqrt(s[:r]))          # [n, r]
        q, a = divmod(h, 4)
        ft_dev[32 * a:32 * a + 32, q, :] = F.T
        gt_dev[32 * a:32 * a + 32, q, :] = G.T
    ft_dev = np.ascontiguousarray(ft_dev).astype(ml_dtypes.bfloat16)
    gt_dev = np.ascontiguousarray(gt_dev).astype(ml_dtypes.bfloat16)

    w_qk = qkv_w[:, :512].copy()
    w_qk[:, :256] *= SCALE
    w_qk_dev = np.ascontiguousarray(
        w_qk.reshape(2, 128, 512).transpose(1, 0, 2)).astype(ml_dtypes.bfloat16)
    w_v_dev = np.ascontiguousarray(
        qkv_w[:, 512:].reshape(2, 128, 256).transpose(1, 0, 2)).astype(
        ml_dtypes.bfloat16)
    w_p_dev = np.ascontiguousarray(
        proj_w.reshape(2, 128, 256).transpose(1, 0, 2)).astype(
        ml_dtypes.bfloat16)
    b_qk = qkv_b[:512].copy()
    b_qk[:256] *= SCALE
    bqk_dev = np.ascontiguousarray(b_qk.reshape(4, 128).T)

    b_v = qkv_b[512:]
    flags = (bool(np.any(b_qk != 0)), bool(np.any(b_v != 0)),
             bool(np.any(proj_b != 0)))

    xt_all = np.ascontiguousarray(
        x.transpose(0, 2, 1).reshape(B_, 2, 128, N).transpose(0, 2, 1, 3)
    ).astype(ml_dtypes.bfloat16)

    common = {"w_qk": w_qk_dev, "w_v": w_v_dev, "w_p": w_p_dev,
              "ft": ft_dev, "gt": gt_dev}
    if flags[0]:
        common["bqk"] = bqk_dev
    if flags[1]:
        common["bv"] = np.ascontiguousarray(b_v)
    if flags[2]:
        common["bp"] = np.ascontiguousarray(proj_b)
    in_maps = []
    for c in range(NCORES):
        m = dict(common)
        m["xt"] = np.ascontiguousarray(xt_all[c * BPC:(c + 1) * BPC])
        in_maps.append(m)
    return in_maps, flags


def kernel(**inputs) -> np.ndarray:
    in_maps, flags = _prep_host(inputs)
    if flags not in _CACHE:
        _CACHE[flags] = _build(flags)
    nc, _ = _CACHE[flags]
    res = run_bass_kernel_spmd(nc, in_maps, core_ids=list(range(NCORES)))
    out = np.concatenate([res.results[c]["out"] for c in range(NCORES)], axis=0)
    return out.astype(np.float32)


def run_traced(**inputs):
    in_maps, flags = _prep_host(inputs)
    if flags not in _CACHE:
        _CACHE[flags] = _build(flags)
    nc, _ = _CACHE[flags]
    res = run_bass_kernel_spmd(nc, in_maps, core_ids=list(range(NCORES)),
                               trace=True)
    out = np.concatenate([res.results[c]["out"] for c in range(NCORES)], axis=0)
    return out.astype(np.float32), res
